# revision 7
# baseline (speedup 1.0000x reference)
"""Trainium2 Bass kernel for nn_AGNN (gnn_message_passing).

Data-parallel over the meta-batch dim B=8: one episode per NeuronCore,
small weights replicated. The whole G+1-stage pipeline (PointSimilarity
pre + G generations of MHA / D2PAgg / PointSimilarity2-with-topk) runs in
a single NEFF per core; the host only shards inputs and stacks outputs.

v2 design notes:
- No Lrelu activation-table (Prelu lives in every table; sigmoid via tanh)
  so ACT never swaps tables.
- psim layer-2 and both d2p layers run as fp8e4 DoubleRow matmuls
  (K=256 in one pass, 0.5 cyc/col); x16 weight scaling folded into the
  next layer / ACT scale.
- Chunk pipeline balanced: GPSIMD does the pairwise subtract, DVE does
  the square (fp16 2x mode) + layer-2 lrelu, ACT does layer-1 lrelu
  (fp8 out) + most of the z-strip copy; strips scatter into zsb by DMA.
- sigmoid(z)*ep_nd = (1+tanh((z+b3)/2))*ep_nd/2; row sums of the final
  renorm are analytic (ep_sum + 1 + 128e-6), removing a reduce chain.
- d2p consumes the un-renormalized masked e ("em") directly (l1norm is
  scale-invariant) so it launches 5+ links earlier.
"""

import sys
import types

sys.path.insert(0, "/root/.axon_site/_ro/trn_rl_repo")
sys.path.insert(0, "/root/.axon_site")

import numpy as np

import concourse.bass as bass
import concourse.tile as tile_mod
from concourse import mybir
from concourse.vector_clock import ScopedClock

# ---------------------------------------------------------------------------
# Patch 1: walrus in this toolchain rejects >1 semaphore wait per
# instruction ("Too many sync wait commands").  Tile freely attaches
# several waits to one instruction.  Split the excess into standalone
# single-wait NoOps placed right before the instruction on its own engine
# (the sequencer executes them in order, so semantics are unchanged).
# ---------------------------------------------------------------------------
_MAX_WAITS = 1


def _split_excess_waits(nc, ordered_instructions_by_block):
    for bb_name, insts in ordered_instructions_by_block.items():
        out = []
        changed = False
        for ins in insts:
            si = ins.sync_info
            ow = list(si.on_wait) if (si is not None and si.on_wait) else []
            if len(ow) > _MAX_WAITS:
                changed = True
                for w in ow[:-_MAX_WAITS]:
                    nop = mybir.InstNoOp(name=nc.get_next_instruction_name())
                    nop.engine = ins.engine
                    nop.sync_info = mybir.SyncInfo(on_wait=[w], on_update=[])
                    nc.register_instruction(nop)
                    out.append(nop)
                ins.sync_info = mybir.SyncInfo(
                    on_wait=ow[-_MAX_WAITS:], on_update=list(si.on_update or [])
                )
            out.append(ins)
        if changed:
            if isinstance(insts, list):
                insts[:] = out
            else:
                ordered_instructions_by_block[bb_name] = out


class _TCWWrapper:
    def __init__(self, tc, ordered_instructions_by_block, **kw):
        self._inner = _RealTCW(tc, ordered_instructions_by_block, **kw)
        self._nc = tc.nc
        self._ordered = ordered_instructions_by_block

    def assign_waits(self, bb_name):
        self._inner.assign_waits(bb_name)
        _split_excess_waits(self._nc, self._ordered)

    def __getattr__(self, k):
        return getattr(self._inner, k)


_RealTCW = tile_mod.TileClockWait
if not getattr(tile_mod, "_ant_wait_split_patched", False):
    tile_mod.TileClockWait = _TCWWrapper
    tile_mod._ant_wait_split_patched = True


def _patched_drain_and_barrier(self, tick_clock, wait_clock):
    nc = self.nc
    drain_inst = nc.sync.drain()
    wait_clock.add_sem_waits(
        drain_inst.ins, ScopedClock({None: tick_clock.global_clock})
    )
    si = drain_inst.ins.sync_info
    ow = list(si.on_wait) if (si is not None and si.on_wait) else []
    if len(ow) > _MAX_WAITS:
        bb = nc.cur_bb.bb
        assert bb.instructions[-1] is drain_inst.ins
        bb.instructions.pop()
        for w in ow[:-_MAX_WAITS]:
            nop = mybir.InstNoOp(name=nc.get_next_instruction_name())
            nop.engine = drain_inst.ins.engine
            nop.sync_info = mybir.SyncInfo(on_wait=[w], on_update=[])
            nc.register_instruction(nop)
            bb.instructions.append(nop)
        drain_inst.ins.sync_info = mybir.SyncInfo(
            on_wait=ow[-_MAX_WAITS:], on_update=list(si.on_update or [])
        )
        bb.instructions.append(drain_inst.ins)

    nc.all_engine_barrier()
    assert self.sems is not None
    popped = nc._tile_sem_poison_stack.pop()
    assert popped is self._sem_poison
    nc.clear_and_free_semaphores(list(self.sems.allocated().values()))
    nc.all_engine_barrier()


if not getattr(tile_mod.TileContext, "_ant_drain_patched", False):
    tile_mod.TileContext._drain_and_barrier = _patched_drain_and_barrier
    tile_mod.TileContext._ant_drain_patched = True


# ---------------------------------------------------------------------------
# Patch 2: NTFF profile hook shim (the image's antenv lacks axon_hooks);
# only needed when run with trace=True, harmless otherwise.
# ---------------------------------------------------------------------------
def _install_ntff_hook():
    if "antenv.axon_hooks" in sys.modules:
        return
    mod = types.ModuleType("antenv.axon_hooks")
    state = {"hook": None}
    mod.set_axon_ntff_profile_hook = lambda h: state.__setitem__("hook", h)
    mod.get_axon_ntff_profile_hook = lambda: state["hook"]
    sys.modules["antenv.axon_hooks"] = mod
    try:
        import antenv

        antenv.axon_hooks = mod
    except ImportError:
        pass
    try:
        from trn_agent_boot.trn_boot import _ntff_profile_via_ctypes

        mod.set_axon_ntff_profile_hook(
            _ntff_profile_via_ctypes("/opt/axon/libaxon_pjrt.so")
        )
    except Exception:
        pass


_install_ntff_hook()

# ---------------------------------------------------------------------------
# Problem constants (hardcoded per spec)
# ---------------------------------------------------------------------------
B, N, C, G, H, DK = 8, 128, 128, 2, 4, 32
BN_SCALE = float(1.0 / np.sqrt(1.0 + 1e-5))
W8SCALE = 16.0          # fp8 weight pre-scale, divided back out downstream
RS_CONST = float(1.0 + N * 1e-6)
F32 = mybir.dt.float32
F16 = mybir.dt.float16
F8 = mybir.dt.float8e4
U8 = mybir.dt.uint8
ALU = mybir.AluOpType
ACT = mybir.ActivationFunctionType
DR = mybir.MatmulPerfMode.DoubleRow
STRIP_ACT_FRAC = 0.78   # fraction of each z-strip copied by ACT (rest DVE)

BLOB16 = [
    ("w1_pre", 2 * C), ("w1_g0", 2 * C), ("w1_g1", 2 * C),
    ("w3_pre", 1), ("w3_g0", 1), ("w3_g1", 1),
    ("ident16", N), ("omeye16", N),
    ("wq_0", H * DK), ("wk_0", H * DK), ("wq_1", H * DK), ("wk_1", H * DK),
]
BLOB8 = [
    ("w2_pre", 2 * C), ("w2_g0", 2 * C), ("w2_g1", 2 * C),
    ("d1_0", 4 * C), ("d1_1", 4 * C),
    ("d2_0", 2 * C), ("d2_1", 2 * C),
]
BLOB32 = [
    ("b3h_pre", 1), ("b3h_g0", 1), ("b3h_g1", 1),
    ("eye_eps", N),
]
CORE16 = [("vpT0", N), ("vprm0", C)]


def _offsets(blob):
    out, off = {}, 0
    for name, w in blob:
        out[name] = (off, w)
        off += w
    return out, off


OFF16, TOT16 = _offsets(BLOB16)
OFF8, TOT8 = _offsets(BLOB8)
OFF32, TOT32 = _offsets(BLOB32)
OFFC16, TOTC16 = _offsets(CORE16)


_BLK = 16


def _tri_chunks():
    """Block-triangular chunk table: (i0, k, j0, w) — rows i0..i0+k,
    cols j0..j0+w, exploiting z's exact symmetry (z[i,j] == z[j,i])."""
    chunks = []
    for b in range(N // _BLK):
        j0 = _BLK * b
        w = N - j0
        kmax = max(1, min(_BLK, 512 // w))
        kmin = max(1, -(-256 // w))  # ceil(256/w)
        rem = _BLK
        i0 = j0
        while rem:
            k = min(kmax, rem)
            if 0 < rem - k < kmin:
                k = rem - kmin
            chunks.append((i0, k, j0, w))
            i0 += k
            rem -= k
    return chunks


_TRI = _tri_chunks()


class _P:
    pass


def _emit_psim_chunks(nc, P, sb, stage, vpT, zsb, hooks):
    """Emit the chunked pair-MLP producing raw-z zsb (fp16, upper blocks).

    hooks: dict {chunk_index: callable} for interleaving overlap work."""
    from concourse.bass_types import AP as _AP

    w1 = P.s16(f"w1_{stage}")
    w2v = P.s8(f"w2_{stage}").rearrange("p (two f) -> p two f", two=2)
    w3 = P.s16(f"w3_{stage}")
    for ci, (i0, k, j0, w) in enumerate(_TRI):
        if ci in hooks:
            hooks[ci]()
        nt = k * w
        a = vpT[:]
        rep = _AP(a.tensor, a.offset + j0, [a.ap[0], [0, k], [1, w]])
        bcast = vpT[:, i0:i0 + k].to_broadcast([C, k, w])
        d = P.work.tile([C, 512], F16, tag="d")
        nc.gpsimd.tensor_tensor(
            out=d[:, 0:nt].rearrange("p (s n) -> p s n", s=k),
            in0=rep, in1=bcast, op=ALU.subtract,
        )
        sim = P.work.tile([C, 512], F16, tag="sim")
        nc.vector.tensor_tensor(out=sim[:, 0:nt], in0=d[:, 0:nt],
                                in1=d[:, 0:nt], op=ALU.mult)
        p1 = P.p1.tile([128, 1024], F32, tag="p1")
        nc.tensor.matmul(p1[:, 0:nt], w1[:, 0:128], sim[:, 0:nt],
                         start=True, stop=True)
        nc.tensor.matmul(p1[:, 512:512 + nt], w1[:, 128:256], sim[:, 0:nt],
                         start=True, stop=True)
        h1 = P.work.tile([128, 1024], F8, tag="h1")
        nc.scalar.activation(
            h1[:, 0:2 * nt].rearrange("p (a n) -> p a n", a=2),
            p1[:].rearrange("p (a n) -> p a n", a=2)[:, :, 0:nt],
            ACT.Prelu, alpha=0.01,
        )
        p2 = P.p2.tile([128, 512], F32, tag="p2")
        nc.tensor.matmul(p2[:, 0:nt], w2v,
                         h1[:, 0:2 * nt].rearrange("p (two n) -> p two n", two=2),
                         start=True, stop=True, perf_mode=DR)
        h2 = P.work.tile([128, 512], F16, tag="h2")
        nc.scalar.activation(h2[:, 0:nt], p2[:, 0:nt], ACT.Prelu, alpha=0.01)
        pz = P.aux.tile([1, 512], F32, tag="pz")
        nc.tensor.matmul(pz[:, 0:nt], w3[:, 0:1], h2[:, 0:nt],
                         start=True, stop=True)
        zl = P.work.tile([1, 512], F16, tag="zl")
        nc.vector.tensor_scalar(out=zl[:, 0:nt], in0=pz[:, 0:nt],
                                scalar1=1.0, scalar2=None, op0=ALU.mult)
        nc.sync.dma_start(zsb[i0:i0 + k, j0:j0 + w], zl[:, 0:nt])


def _emit_epilogue(nc, P, sb, stage, zsb, ctx, nb):
    """zsb(raw upper) -> mirror merge -> th -> e -> (topk) -> em, s1.

    ctx: (ep_nd_half f16, shalf f32 [N,1], r2 f32 [N,1])."""
    ep_nd_half, shalf, r2 = ctx
    pT = P.aux.tile([128, 1024], F16, tag="pz")
    nc.tensor.transpose(pT[:, 0:128], zsb[:], P.ident16[:])
    nc.vector.copy_predicated(zsb[:], P.masklow[:], pT[:, 0:128])
    th = sb.tile([N, N], F16, tag="th")
    nc.scalar.activation(th[:], zsb[:], ACT.Tanh, scale=0.5,
                         bias=P.s32(f"b3h_{stage}")[:, 0:1])
    e = sb.tile([N, N], F16, tag="e")
    s1 = sb.tile([N, 1], F32, tag="s1")
    if nb:
        nc.vector.scalar_tensor_tensor(out=e[:], in0=th[:], scalar=1.0,
                                       op0=ALU.add, in1=ep_nd_half[:],
                                       op1=ALU.mult)
        x = sb.tile([N, N], F32, tag="topk_x")
        nc.vector.tensor_scalar(out=x[:], in0=e[:], scalar1=-1.0, scalar2=2.0,
                                op0=ALU.mult, op1=ALU.add)
        mx = sb.tile([N, 8], F32, tag="topk_mx")
        done = 0
        while done < nb:
            kk = min(8, nb - done)
            nc.vector.max(out=mx[:], in_=x[:])
            if kk < 8:
                nc.vector.memset(mx[:, kk:], 0.0)
            nc.vector.match_replace(out=x[:], in_to_replace=mx[:],
                                    in_values=x[:], imm_value=0.0)
            done += kk
        m = sb.tile([N, N], F16, tag="topk_m")
        nc.vector.tensor_scalar(out=m[:], in0=x[:], scalar1=0.5, scalar2=None,
                                op0=ALU.is_gt)
        em = sb.tile([N, N], F16, tag=f"em_{stage}")
        nc.vector.scalar_tensor_tensor(out=em[:], in0=e[:], scalar=1.0,
                                       op0=ALU.bypass, in1=m[:], op1=ALU.mult,
                                       accum_out=s1[:])
    else:
        em = e
        nc.vector.scalar_tensor_tensor(out=e[:], in0=th[:], scalar=1.0,
                                       op0=ALU.add, in1=ep_nd_half[:],
                                       op1=ALU.mult, accum_out=s1[:])
    # v = shalf/(s1*rs) where rs = 2*shalf + RS_CONST;  f = 2*v
    rinv = sb.tile([N, 1], F32, tag="rinv")
    nc.vector.reciprocal(rinv[:], s1[:])
    t1 = sb.tile([N, 1], F32, tag="vt1")
    nc.vector.tensor_tensor(out=t1[:], in0=shalf[:], in1=rinv[:], op=ALU.mult)
    v = sb.tile([N, 1], F32, tag="v")
    nc.vector.tensor_tensor(out=v[:], in0=t1[:], in1=r2[:], op=ALU.mult)
    return em, v


def _emit_mha_scores(nc, P, sb, g, vpT):
    pqk = P.aux.tile([128, 512], F32, tag="pz")
    nc.tensor.matmul(pqk[:, 0:128], P.s16(f"wq_{g}")[:], vpT[:],
                     start=True, stop=True)
    nc.tensor.matmul(pqk[:, 128:256], P.s16(f"wk_{g}")[:], vpT[:],
                     start=True, stop=True)
    qkT = sb.tile([C, 2 * N], F16, tag=f"qkT_{g}")
    nc.vector.tensor_scalar(out=qkT[:], in0=pqk[:, 0:256], scalar1=1.0,
                            scalar2=None, op0=ALU.mult)
    qkh = sb.tile([DK, H, 2 * N], F16, tag=f"qkh_{g}")
    for h in range(H):
        nc.sync.dma_start(qkh[0:DK, h, :], qkT[DK * h:DK * (h + 1), :])
    ps2 = P.aux.tile([128, 512], F32, tag="pz")
    for h in range(H):
        nc.tensor.matmul(ps2[:, 128 * h:128 * (h + 1)],
                         qkh[0:DK, h, 0:N], qkh[0:DK, h, N:2 * N],
                         start=True, stop=True)
    ssb = sb.tile([128, 512], F16, tag=f"mha_s_{g}")
    nc.vector.tensor_scalar(out=ssb[:], in0=ps2[:], scalar1=1.0, scalar2=None,
                            op0=ALU.mult)
    inv_sqrt_dk = float(1.0 / np.sqrt(DK))
    mxs = sb.tile([N, H], F32, tag=f"mha_mx_{g}")
    nbias = sb.tile([N, H], F32, tag=f"mha_nb_{g}")
    nc.vector.tensor_reduce(
        mxs[:], ssb[:].rearrange("p (h n) -> p h n", h=H),
        axis=mybir.AxisListType.X, op=ALU.max)
    nc.vector.tensor_scalar(out=nbias[:], in0=mxs[:], scalar1=-inv_sqrt_dk,
                            scalar2=None, op0=ALU.mult)
    return ssb, nbias


def _emit_mha_softmax(nc, P, sb, g, ssb, nbias):
    inv_sqrt_dk = float(1.0 / np.sqrt(DK))
    ehs, ses = [], []
    for h in range(H):
        eh = sb.tile([N, N], F16, tag=f"mha_eh{h}")
        se = sb.tile([N, 1], F32, tag=f"mha_se{h}")
        nc.scalar.activation(eh[:], ssb[:, 128 * h:128 * (h + 1)], ACT.Exp,
                             bias=nbias[:, h:h + 1], scale=inv_sqrt_dk,
                             accum_out=se[:])
        ehs.append(eh)
        ses.append(se)
    rcs = []
    for h in range(H):
        rc = sb.tile([N, 1], F32, tag=f"mha_rc{h}")
        nc.vector.reciprocal(rc[:], ses[h][:])
        rcs.append(rc)
    t1 = sb.tile([N, N], F16, tag="mha_t1")
    nc.vector.tensor_scalar(out=t1[:], in0=ehs[1][:], scalar1=rcs[1][:],
                            scalar2=None, op0=ALU.mult)
    a01 = sb.tile([N, N], F16, tag="mha_a01")
    nc.vector.scalar_tensor_tensor(out=a01[:], in0=ehs[0][:], scalar=rcs[0][:],
                                   op0=ALU.mult, in1=t1[:], op1=ALU.add)
    t2 = sb.tile([N, N], F16, tag="mha_t2")
    nc.vector.tensor_scalar(out=t2[:], in0=ehs[3][:], scalar1=rcs[3][:],
                            scalar2=None, op0=ALU.mult)
    a23 = sb.tile([N, N], F16, tag="mha_a23")
    nc.vector.scalar_tensor_tensor(out=a23[:], in0=ehs[2][:], scalar=rcs[2][:],
                                   op0=ALU.mult, in1=t2[:], op1=ALU.add)
    attn = sb.tile([N, N], F16, tag=f"attn_{g}")
    nc.vector.tensor_tensor(out=attn[:], in0=a01[:], in1=a23[:], op=ALU.add)
    # no diagonal kill needed: em's diagonal is 0, so edge = em*attn has it too
    return attn


def _emit_d2p(nc, P, sb, g, em, attn, x8, vp_rm, last):
    """vpT_new = D2PAgg(em-normalized edges, vp); x8 holds vpT fp8 in
    [:, 0:N] already. Returns (vpT_new f16, x8_next, vp_rm_new or None)."""
    edge = sb.tile([N, N], F16, tag="edge")
    s = sb.tile([N, 1], F32, tag="d2p_s")
    nc.vector.scalar_tensor_tensor(out=edge[:], in0=em[:], scalar=1.0,
                                   op0=ALU.bypass, in1=attn[:], op1=ALU.mult,
                                   accum_out=s[:])
    rr = sb.tile([N, 1], F32, tag="d2p_rr")
    nc.vector.reciprocal(rr[:], s[:])
    diagm = sb.tile([N, N], F16, tag="diagm")
    nc.vector.tensor_scalar(out=diagm[:], in0=P.ident16[:], scalar1=rr[:],
                            scalar2=None, op0=ALU.mult)
    pTe = P.aux.tile([128, 512], F32, tag="pz")
    nc.tensor.matmul(pTe[:, 0:128], edge[:], diagm[:], start=True, stop=True)
    edgeTs = sb.tile([N, N], F16, tag="edgeTs")
    nc.vector.tensor_scalar(out=edgeTs[:], in0=pTe[:, 0:128], scalar1=1.0,
                            scalar2=None, op0=ALU.mult)
    pa = P.aux.tile([128, 512], F32, tag="pz")
    nc.tensor.matmul(pa[:, 0:128], vp_rm[:], edgeTs[:], start=True, stop=True)
    nc.vector.tensor_scalar(out=x8[:, C:2 * C], in0=pa[:, 0:128], scalar1=1.0,
                            scalar2=None, op0=ALU.mult)
    d1 = P.s8(f"d1_{g}")
    pm = P.aux.tile([128, 512], F32, tag="pz")
    x8v = x8[:].rearrange("p (two n) -> p two n", two=2)
    for oh in range(2):
        nc.tensor.matmul(pm[:, 128 * oh:128 * (oh + 1)],
                         d1[:, 256 * oh:256 * (oh + 1)].rearrange(
                             "p (two f) -> p two f", two=2),
                         x8v, start=True, stop=True, perf_mode=DR)
    h8 = sb.tile([128, 2 * C], F8, tag="d2p_h8")
    nc.scalar.activation(h8[:], pm[:, 0:256], ACT.Prelu, alpha=0.01,
                         scale=1.0 / W8SCALE)
    pv = P.aux.tile([128, 512], F32, tag="pz")
    nc.tensor.matmul(pv[:, 0:128],
                     P.s8(f"d2_{g}").rearrange("p (two f) -> p two f", two=2),
                     h8[:].rearrange("p (two n) -> p two n", two=2),
                     start=True, stop=True, perf_mode=DR)
    vpT_new = sb.tile([C, N], F16, tag=f"vpT_{g + 1}")
    nc.scalar.activation(vpT_new[:], pv[:, 0:128], ACT.Prelu, alpha=0.01,
                         scale=1.0 / W8SCALE)
    x8_next = None
    vp_rm_new = None
    if not last:
        x8_next = sb.tile([128, 2 * C], F8, tag=f"x8_{g + 1}")
        nc.vector.tensor_scalar(out=x8_next[:, 0:C], in0=vpT_new[:],
                                scalar1=1.0, scalar2=None, op0=ALU.mult)
        pt2 = P.aux.tile([128, 1024], F16, tag="pz")
        nc.tensor.transpose(pt2[:, 0:128], vpT_new[:], P.ident16[:])
        vp_rm_new = sb.tile([N, C], F16, tag=f"vprm_{g + 1}")
        nc.vector.tensor_scalar(out=vp_rm_new[:], in0=pt2[:, 0:128],
                                scalar1=1.0, scalar2=None, op0=ALU.mult)
    return vpT_new, x8_next, vp_rm_new


def build():
    """Build the single-core Bass graph (SPMD across 8 cores)."""
    nc = bass.Bass()
    c16_ext = nc.declare_dram_parameter("core16", [128, TOTC16], F16,
                                        isOutput=False)
    b16_ext = nc.declare_dram_parameter("blob16", [128, TOT16], F16,
                                        isOutput=False)
    b8_ext = nc.declare_dram_parameter("blob8", [128, TOT8], U8,
                                       isOutput=False)
    c32_ext = nc.declare_dram_parameter("core32", [128, N], F32,
                                        isOutput=False)
    b32_ext = nc.declare_dram_parameter("blob32", [128, TOT32], F32,
                                        isOutput=False)
    mask_ext = nc.declare_dram_parameter("masklow", [N, N], U8, isOutput=False)
    out_ext = nc.declare_dram_parameter("out", [N, N], F32, isOutput=True)

    with tile_mod.TileContext(nc) as tc:
        with tc.tile_pool(name="const", bufs=1) as const, \
             tc.tile_pool(name="state", bufs=1) as state, \
             tc.tile_pool(name="work", bufs=3) as work, \
             tc.tile_pool(name="p1", bufs=2, space="PSUM") as p1, \
             tc.tile_pool(name="p2", bufs=2, space="PSUM") as p2, \
             tc.tile_pool(name="pz", bufs=2, space="PSUM") as pz:

            P = _P()
            P.work, P.p1, P.p2, P.aux = work, p1, p2, pz

            c16 = const.tile([128, TOTC16], F16, tag="c16")
            nc.sync.dma_start(c16[:], c16_ext[:])
            b16 = const.tile([128, TOT16], F16, tag="b16")
            nc.sync.dma_start(b16[:], b16_ext[:])
            b8 = const.tile([128, TOT8], U8, tag="b8")
            nc.sync.dma_start(b8[:], b8_ext[:])
            c32 = const.tile([128, N], F32, tag="c32")
            nc.gpsimd.dma_start(c32[:], c32_ext[:])
            b32 = const.tile([128, TOT32], F32, tag="b32")
            nc.gpsimd.dma_start(b32[:], b32_ext[:])
            P.masklow = const.tile([N, N], U8, tag="masklow")
            nc.gpsimd.dma_start(P.masklow[:], mask_ext[:])

            def s16(name):
                off, wd = OFF16[name]
                return b16[:, off:off + wd]

            def s8(name):
                off, wd = OFF8[name]
                return b8[:, off:off + wd].bitcast(F8)

            def s32(name):
                off, wd = OFF32[name]
                return b32[:, off:off + wd]

            P.s16, P.s8, P.s32 = s16, s8, s32
            P.ident16 = s16("ident16")
            P.omeye16 = s16("omeye16")

            vpT0 = c16[:, OFFC16["vpT0"][0]:OFFC16["vpT0"][0] + N]
            vprm0 = c16[:, OFFC16["vprm0"][0]:OFFC16["vprm0"][0] + C]
            ep0 = c32[:]

            # ---------------- stage pre ----------------
            zsb = state.tile([N, N], F16, tag="zsb_pre")
            ep_nd_half = state.tile([N, N], F16, tag="ephalf_pre")
            shalf = state.tile([N, 1], F32, tag="shalf_pre")
            r2 = state.tile([N, 1], F32, tag="r2_pre")
            x8_0 = state.tile([128, 2 * C], F8, tag="x8_0")

            scores = {}

            def pre_hook2():
                # ep-derived context (waits on the later c32 DMA; emitted
                # after chunk 2 to avoid head-of-line blocking the DVE)
                nc.vector.scalar_tensor_tensor(
                    out=ep_nd_half[:], in0=ep0[:], scalar=0.5, op0=ALU.mult,
                    in1=P.omeye16[:], op1=ALU.mult, accum_out=shalf[:])
                rr = P.work.tile([N, 1], F32, tag="rrtmp")
                nc.vector.tensor_scalar(out=rr[:], in0=shalf[:], scalar1=2.0,
                                        scalar2=RS_CONST, op0=ALU.mult,
                                        op1=ALU.add)
                nc.vector.reciprocal(r2[:], rr[:])
                nc.vector.tensor_scalar(out=x8_0[:, 0:C], in0=vpT0,
                                        scalar1=1.0, scalar2=None, op0=ALU.mult)

            def pre_hook4():
                scores[0] = _emit_mha_scores(nc, P, state, 0, vpT0)

            def pre_hook10():
                scores["attn0"] = _emit_mha_softmax(nc, P, state, 0,
                                                    *scores[0])

            _emit_psim_chunks(nc, P, state, "pre", vpT0, zsb,
                              {2: pre_hook2, 4: pre_hook4, 10: pre_hook10})
            em, v = _emit_epilogue(nc, P, state, "pre", zsb,
                                   (ep_nd_half, shalf, r2), 0)

            vpT1, x8_1, vprm1 = _emit_d2p(nc, P, state, 0, em,
                                          scores["attn0"], x8_0, vprm0,
                                          last=False)
            # next-stage ep context from em (off critical path)
            ctx_g0 = _next_ctx(nc, P, state, "g0", em, v)

            # ---------------- stage g0 ----------------
            zsb1 = state.tile([N, N], F16, tag="zsb_g0")

            def g0_hook4():
                scores[1] = _emit_mha_scores(nc, P, state, 1, vpT1)

            def g0_hook10():
                scores["attn1"] = _emit_mha_softmax(nc, P, state, 1,
                                                    *scores[1])

            _emit_psim_chunks(nc, P, state, "g0", vpT1, zsb1,
                              {4: g0_hook4, 10: g0_hook10})
            nb0 = N - int(N * 0.9)
            em0, v0 = _emit_epilogue(nc, P, state, "g0", zsb1, ctx_g0, nb0)
            vpT2, _, _ = _emit_d2p(nc, P, state, 1, em0, scores["attn1"],
                                   x8_1, vprm1, last=True)
            ctx_g1 = _next_ctx(nc, P, state, "g1", em0, v0)

            # ---------------- stage g1 ----------------
            zsb2 = state.tile([N, N], F16, tag="zsb_g1")

            eyeg = state.tile([N, N], F32, tag="eyeg")

            def g1_hook4():
                nc.vector.tensor_scalar(out=eyeg[:], in0=P.s32("eye_eps")[:],
                                        scalar1=ctx_g1[2][:], scalar2=None,
                                        op0=ALU.mult)

            _emit_psim_chunks(nc, P, state, "g1", vpT2, zsb2, {4: g1_hook4})
            nb1 = N - int(N * 0.8)
            em1, v1 = _emit_epilogue(nc, P, state, "g1", zsb2, ctx_g1, nb1)
            f = state.tile([N, 1], F32, tag="ffin")
            nc.vector.tensor_scalar(out=f[:], in0=v1[:], scalar1=2.0,
                                    scalar2=None, op0=ALU.mult)
            outt = state.tile([N, N], F32, tag="outt")
            nc.vector.scalar_tensor_tensor(out=outt[:], in0=em1[:],
                                           scalar=f[:], op0=ALU.mult,
                                           in1=eyeg[:], op1=ALU.add)
            nc.sync.dma_start(out_ext[:], outt[:])

    return nc


def _next_ctx(nc, P, state, stage, em, v):
    """ep context for the next psim stage, derived from em and v = f/2."""
    ephn = state.tile([N, N], F16, tag=f"ephalf_{stage}")
    shn = state.tile([N, 1], F32, tag=f"shalf_{stage}")
    nc.vector.tensor_scalar(out=ephn[:], in0=em[:], scalar1=v[:],
                            scalar2=0.0, op0=ALU.mult, op1=ALU.add,
                            accum_out=shn[:])
    rr = state.tile([N, 1], F32, tag=f"rr_{stage}")
    nc.vector.tensor_scalar(out=rr[:], in0=shn[:], scalar1=2.0,
                            scalar2=RS_CONST, op0=ALU.mult, op1=ALU.add)
    r2n = state.tile([N, 1], F32, tag=f"r2_{stage}")
    nc.vector.reciprocal(r2n[:], rr[:])
    return (ephn, shn, r2n)


def make_in_maps(vp, ep0, ps_pre_w1, ps_pre_w2, ps_pre_w3, ps_pre_b3,
                 ps_w1, ps_w2, ps_w3, ps_b3, d2p_w1, d2p_w2, mha_wq, mha_wk):
    import ml_dtypes

    f = np.float32
    a = lambda x: np.ascontiguousarray(np.asarray(x), dtype=f)
    eye = np.eye(N, dtype=f)
    ii = np.arange(N)
    masklow = np.ascontiguousarray(
        (ii[None, :] < (ii[:, None] // _BLK) * _BLK).astype(np.uint8))

    def pack_dr(wmat):
        """[2C, M] -> [128, 2*M] fp8 bytes in DoubleRow [p, 2, M] layout."""
        wm = np.asarray(wmat, dtype=f)
        ktiles, m = 2, wm.shape[1]
        outb = np.zeros((128, 2 * m), dtype=f)
        for i in range(ktiles):
            outb[:, i * m:(i + 1) * m] = wm[i * 128:(i + 1) * 128, :]
        # interleave k-tiles along dim1 as [p, 2, m] contiguous:
        # layout [p, (two m)] with two-major is exactly the above? No:
        # [p, two, m] means two-major blocks of m — which is what we built.
        return outb

    vals16 = {"ident16": eye, "omeye16": a(1.0 - eye)}
    vals32 = {"eye_eps": a(eye + 1e-6)}
    vals8 = {}
    for s, w1, w2, w3, b3 in [
        ("pre", ps_pre_w1, ps_pre_w2, ps_pre_w3, ps_pre_b3),
        ("g0", ps_w1[0], ps_w2[0], ps_w3[0], ps_b3[0]),
        ("g1", ps_w1[1], ps_w2[1], ps_w3[1], ps_b3[1]),
    ]:
        vals16[f"w1_{s}"] = a(np.asarray(w1) * BN_SCALE)
        vals8[f"w2_{s}"] = pack_dr(a(np.asarray(w2) * BN_SCALE * W8SCALE))
        vals16[f"w3_{s}"] = a(np.asarray(w3) / W8SCALE)
        vals32[f"b3h_{s}"] = a(np.broadcast_to(
            np.asarray(b3).reshape(1, 1) * 0.5, (N, 1)))
    for g in range(G):
        d1s = a(np.asarray(d2p_w1[g]) * BN_SCALE * W8SCALE)   # [2C, 2C]
        d2s = a(np.asarray(d2p_w2[g]) * BN_SCALE * W8SCALE)   # [2C, C]
        vals8[f"d1_{g}"] = np.concatenate(
            [pack_dr(d1s[:, 0:128]), pack_dr(d1s[:, 128:256])], axis=1)
        vals8[f"d2_{g}"] = pack_dr(d2s)
        vals16[f"wq_{g}"] = a(mha_wq[g])
        vals16[f"wk_{g}"] = a(mha_wk[g])

    blob16 = np.zeros((128, TOT16), dtype=np.float16)
    for name, wd in BLOB16:
        off = OFF16[name][0]
        blob16[:, off:off + wd] = vals16[name].astype(np.float16)
    blob8 = np.zeros((128, TOT8), dtype=np.uint8)
    for name, wd in BLOB8:
        off = OFF8[name][0]
        blob8[:, off:off + wd] = vals8[name].astype(
            ml_dtypes.float8_e4m3).view(np.uint8)
    blob32 = np.zeros((128, TOT32), dtype=f)
    for name, wd in BLOB32:
        off = OFF32[name][0]
        blob32[:, off:off + wd] = vals32[name]

    shared = {"blob16": blob16, "blob8": blob8, "blob32": blob32,
              "masklow": masklow}
    vp = a(vp)
    ep0 = a(ep0)
    in_maps = []
    for i in range(B):
        c16 = np.zeros((128, TOTC16), dtype=np.float16)
        c16[:, OFFC16["vpT0"][0]:OFFC16["vpT0"][0] + N] = vp[i].T
        c16[:, OFFC16["vprm0"][0]:OFFC16["vprm0"][0] + C] = vp[i]
        m = dict(shared)
        m["core16"] = c16
        m["core32"] = ep0[i]
        in_maps.append(m)
    return in_maps


_CACHED_NC = None


def _get_nc():
    global _CACHED_NC
    if _CACHED_NC is None:
        _CACHED_NC = build()
    return _CACHED_NC


def run(in_maps, trace=False):
    from concourse.bass_utils import run_bass_kernel_spmd

    nc = _get_nc()
    return run_bass_kernel_spmd(nc, in_maps, list(range(B)), trace=trace)


def kernel(**inputs) -> np.ndarray:
    in_maps = make_in_maps(**inputs)
    r = run(in_maps, trace=False)
    return np.stack([r.results[i]["out"] for i in range(B)]).astype(np.float32)


# revision 17
# speedup vs baseline: 1.1326x; 1.1326x over previous
"""Trainium2 Bass kernel for nn_AGNN (gnn_message_passing).

Data-parallel over the meta-batch dim B=8: one episode per NeuronCore,
small weights replicated. The whole G+1-stage pipeline (PointSimilarity
pre + G generations of MHA / D2PAgg / PointSimilarity2-with-topk) runs in
a single NEFF per core; the host only shards inputs and stacks outputs.

v2 design notes:
- No Lrelu activation-table (Prelu lives in every table; sigmoid via tanh)
  so ACT never swaps tables.
- psim layer-2 and both d2p layers run as fp8e4 DoubleRow matmuls
  (K=256 in one pass, 0.5 cyc/col); x16 weight scaling folded into the
  next layer / ACT scale.
- Chunk pipeline balanced: GPSIMD does the pairwise subtract, DVE does
  the square (fp16 2x mode) + layer-2 lrelu, ACT does layer-1 lrelu
  (fp8 out) + most of the z-strip copy; strips scatter into zsb by DMA.
- sigmoid(z)*ep_nd = (1+tanh((z+b3)/2))*ep_nd/2; row sums of the final
  renorm are analytic (ep_sum + 1 + 128e-6), removing a reduce chain.
- d2p consumes the un-renormalized masked e ("em") directly (l1norm is
  scale-invariant) so it launches 5+ links earlier.
"""

import sys
import types

sys.path.insert(0, "/root/.axon_site/_ro/trn_rl_repo")
sys.path.insert(0, "/root/.axon_site")

import numpy as np

import concourse.bass as bass
import concourse.tile as tile_mod
from concourse import mybir
from concourse.vector_clock import ScopedClock

# ---------------------------------------------------------------------------
# Patch 1: walrus in this toolchain rejects >1 semaphore wait per
# instruction ("Too many sync wait commands").  Tile freely attaches
# several waits to one instruction.  Split the excess into standalone
# single-wait NoOps placed right before the instruction on its own engine
# (the sequencer executes them in order, so semantics are unchanged).
# ---------------------------------------------------------------------------
_MAX_WAITS = 1


def _split_excess_waits(nc, ordered_instructions_by_block):
    for bb_name, insts in ordered_instructions_by_block.items():
        out = []
        changed = False
        for ins in insts:
            si = ins.sync_info
            ow = list(si.on_wait) if (si is not None and si.on_wait) else []
            if len(ow) > _MAX_WAITS:
                changed = True
                for w in ow[:-_MAX_WAITS]:
                    nop = mybir.InstNoOp(name=nc.get_next_instruction_name())
                    nop.engine = ins.engine
                    nop.sync_info = mybir.SyncInfo(on_wait=[w], on_update=[])
                    nc.register_instruction(nop)
                    out.append(nop)
                ins.sync_info = mybir.SyncInfo(
                    on_wait=ow[-_MAX_WAITS:], on_update=list(si.on_update or [])
                )
            out.append(ins)
        if changed:
            if isinstance(insts, list):
                insts[:] = out
            else:
                ordered_instructions_by_block[bb_name] = out


class _TCWWrapper:
    def __init__(self, tc, ordered_instructions_by_block, **kw):
        self._inner = _RealTCW(tc, ordered_instructions_by_block, **kw)
        self._nc = tc.nc
        self._ordered = ordered_instructions_by_block

    def assign_waits(self, bb_name):
        self._inner.assign_waits(bb_name)
        _split_excess_waits(self._nc, self._ordered)

    def __getattr__(self, k):
        return getattr(self._inner, k)


_RealTCW = tile_mod.TileClockWait
if not getattr(tile_mod, "_ant_wait_split_patched", False):
    tile_mod.TileClockWait = _TCWWrapper
    tile_mod._ant_wait_split_patched = True


def _patched_drain_and_barrier(self, tick_clock, wait_clock):
    nc = self.nc
    drain_inst = nc.sync.drain()
    wait_clock.add_sem_waits(
        drain_inst.ins, ScopedClock({None: tick_clock.global_clock})
    )
    si = drain_inst.ins.sync_info
    ow = list(si.on_wait) if (si is not None and si.on_wait) else []
    if len(ow) > _MAX_WAITS:
        bb = nc.cur_bb.bb
        assert bb.instructions[-1] is drain_inst.ins
        bb.instructions.pop()
        for w in ow[:-_MAX_WAITS]:
            nop = mybir.InstNoOp(name=nc.get_next_instruction_name())
            nop.engine = drain_inst.ins.engine
            nop.sync_info = mybir.SyncInfo(on_wait=[w], on_update=[])
            nc.register_instruction(nop)
            bb.instructions.append(nop)
        drain_inst.ins.sync_info = mybir.SyncInfo(
            on_wait=ow[-_MAX_WAITS:], on_update=list(si.on_update or [])
        )
        bb.instructions.append(drain_inst.ins)

    nc.all_engine_barrier()
    assert self.sems is not None
    popped = nc._tile_sem_poison_stack.pop()
    assert popped is self._sem_poison
    nc.clear_and_free_semaphores(list(self.sems.allocated().values()))
    nc.all_engine_barrier()


if not getattr(tile_mod.TileContext, "_ant_drain_patched", False):
    tile_mod.TileContext._drain_and_barrier = _patched_drain_and_barrier
    tile_mod.TileContext._ant_drain_patched = True


# ---------------------------------------------------------------------------
# Patch 2: NTFF profile hook shim (the image's antenv lacks axon_hooks);
# only needed when run with trace=True, harmless otherwise.
# ---------------------------------------------------------------------------
def _install_ntff_hook():
    if "antenv.axon_hooks" in sys.modules:
        return
    mod = types.ModuleType("antenv.axon_hooks")
    state = {"hook": None}
    mod.set_axon_ntff_profile_hook = lambda h: state.__setitem__("hook", h)
    mod.get_axon_ntff_profile_hook = lambda: state["hook"]
    sys.modules["antenv.axon_hooks"] = mod
    try:
        import antenv

        antenv.axon_hooks = mod
    except ImportError:
        pass
    try:
        from trn_agent_boot.trn_boot import _ntff_profile_via_ctypes

        mod.set_axon_ntff_profile_hook(
            _ntff_profile_via_ctypes("/opt/axon/libaxon_pjrt.so")
        )
    except Exception:
        pass


_install_ntff_hook()

# ---------------------------------------------------------------------------
# Problem constants (hardcoded per spec)
# ---------------------------------------------------------------------------
B, N, C, G, H, DK = 8, 128, 128, 2, 4, 32
BN_SCALE = float(1.0 / np.sqrt(1.0 + 1e-5))
W8SCALE = 16.0          # fp8 weight pre-scale, divided back out downstream
RS_CONST = float(1.0 + N * 1e-6)
F32 = mybir.dt.float32
F16 = mybir.dt.float16
F8 = mybir.dt.float8e4
U8 = mybir.dt.uint8
ALU = mybir.AluOpType
ACT = mybir.ActivationFunctionType
DR = mybir.MatmulPerfMode.DoubleRow
STRIP_ACT_FRAC = 0.78   # fraction of each z-strip copied by ACT (rest DVE)

BLOB16 = [
    ("w1_pre", 2 * C), ("w1_g0", 2 * C), ("w1_g1", 2 * C),
    ("w2a_pre", C), ("w2b_pre", C), ("w2a_g0", C), ("w2b_g0", C),
    ("w2a_g1", C), ("w2b_g1", C),
    ("w3_pre", 1), ("w3_g0", 1), ("w3_g1", 1),
    ("ident16", N), ("omeye16", N),
    ("wq_0", H * DK), ("wk_0", H * DK), ("wq_1", H * DK), ("wk_1", H * DK),
    ("d1k0_0", 2 * C), ("d1k1_0", 2 * C), ("d2k0_0", C), ("d2k1_0", C),
    ("d1k0_1", 2 * C), ("d1k1_1", 2 * C), ("d2k0_1", C), ("d2k1_1", C),
]
BLOB32 = [
    ("b3h_pre", 1), ("b3h_g0", 1), ("b3h_g1", 1),
    ("eye_eps", N),
]
CORE16 = [("vpT0", N), ("vprm0", C)]


def _offsets(blob):
    out, off = {}, 0
    for name, w in blob:
        out[name] = (off, w)
        off += w
    return out, off


OFF16, TOT16 = _offsets(BLOB16)
OFF32, TOT32 = _offsets(BLOB32)
OFFC16, TOTC16 = _offsets(CORE16)


_BLK = 16


def _tri_chunks():
    """Block-triangular chunk table: (i0, k, j0, w) — rows i0..i0+k,
    cols j0..j0+w, exploiting z's exact symmetry (z[i,j] == z[j,i])."""
    chunks = []
    for b in range(N // _BLK):
        j0 = _BLK * b
        w = N - j0
        kmax = max(1, min(_BLK, 512 // w))
        kmin = max(1, -(-256 // w))  # ceil(256/w)
        rem = _BLK
        i0 = j0
        while rem:
            k = min(kmax, rem)
            if 0 < rem - k < kmin:
                k = rem - kmin
            chunks.append((i0, k, j0, w))
            i0 += k
            rem -= k
    return chunks


_TRI = _tri_chunks()


class _P:
    pass


def _emit_psim_chunks(nc, P, sb, stage, vpT, zsb, hooks):
    """Emit the chunked pair-MLP producing raw-z zsb (fp16, upper blocks).

    hooks: dict {chunk_index: callable} for interleaving overlap work."""
    from concourse.bass_types import AP as _AP

    w1 = P.s16(f"w1_{stage}")
    w2a = P.s16(f"w2a_{stage}")
    w2b = P.s16(f"w2b_{stage}")
    w3 = P.s16(f"w3_{stage}")
    for ci, (i0, k, j0, w) in enumerate(_TRI):
        if ci in hooks:
            hooks[ci]()
        nt = k * w
        a = vpT[:]
        rep = _AP(a.tensor, a.offset + j0, [a.ap[0], [0, k], [1, w]])
        bcast = vpT[:, i0:i0 + k].to_broadcast([C, k, w])
        d = P.work.tile([C, 512], F16, tag="d")
        nc.vector.tensor_tensor(
            out=d[:, 0:nt].rearrange("p (s n) -> p s n", s=k),
            in0=rep, in1=bcast, op=ALU.subtract,
        )
        sim = P.work.tile([C, 512], F16, tag="sim")
        nc.gpsimd.tensor_tensor(out=sim[:, 0:nt], in0=d[:, 0:nt],
                                in1=d[:, 0:nt], op=ALU.mult)
        p1 = P.p1.tile([128, 1024], F32, tag="p1")
        nc.tensor.matmul(p1[:, 0:nt], w1[:, 0:128], sim[:, 0:nt],
                         start=True, stop=True)
        nc.tensor.matmul(p1[:, 512:512 + nt], w1[:, 128:256], sim[:, 0:nt],
                         start=True, stop=True)
        h1 = P.work.tile([128, 1024], F16, tag="h1")
        nc.scalar.activation(
            h1[:, 0:2 * nt].rearrange("p (a n) -> p a n", a=2),
            p1[:].rearrange("p (a n) -> p a n", a=2)[:, :, 0:nt],
            ACT.Prelu, alpha=0.01,
        )
        p2 = P.p2.tile([128, 512], F32, tag="p2")
        nc.tensor.matmul(p2[:, 0:nt], w2a[:], h1[:, 0:nt],
                         start=True, stop=False)
        nc.tensor.matmul(p2[:, 0:nt], w2b[:], h1[:, nt:2 * nt],
                         start=False, stop=True)
        h2 = P.work.tile([128, 512], F16, tag="h2")
        nc.scalar.activation(h2[:, 0:nt], p2[:, 0:nt], ACT.Prelu, alpha=0.01)
        pz = P.aux.tile([1, 512], F32, tag="pz")
        nc.tensor.matmul(pz[:, 0:nt], w3[:, 0:1], h2[:, 0:nt],
                         start=True, stop=True)
        zl = P.work.tile([1, 512], F16, tag="zl")
        nc.vector.tensor_scalar(out=zl[:, 0:nt], in0=pz[:, 0:nt],
                                scalar1=1.0, scalar2=None, op0=ALU.mult)
        nc.sync.dma_start(zsb[i0:i0 + k, j0:j0 + w], zl[:, 0:nt])


def _emit_epilogue(nc, P, sb, stage, zsb, ctx, nb):
    """zsb(raw upper) -> mirror merge -> th -> e -> (topk) -> em, s1.

    ctx: (ep_nd_half f16, shalf f32 [N,1], r2 f32 [N,1])."""
    ep_nd_half, shalf, r2 = ctx
    pT = P.aux.tile([128, 1024], F16, tag="pz")
    nc.tensor.transpose(pT[:, 0:128], zsb[:], P.ident16[:])
    nc.vector.copy_predicated(zsb[:], P.masklow[:], pT[:, 0:128])
    th = sb.tile([N, N], F16, tag="th")
    nc.scalar.activation(th[:], zsb[:], ACT.Tanh, scale=0.5,
                         bias=P.s32(f"b3h_{stage}")[:, 0:1])
    e = sb.tile([N, N], F16, tag="e")
    s1 = sb.tile([N, 1], F32, tag="s1")
    if nb:
        nc.vector.scalar_tensor_tensor(out=e[:], in0=th[:], scalar=1.0,
                                       op0=ALU.add, in1=ep_nd_half[:],
                                       op1=ALU.mult)
        x = sb.tile([N, N], F32, tag="topk_x")
        nc.vector.tensor_scalar(out=x[:], in0=e[:], scalar1=-1.0, scalar2=2.0,
                                op0=ALU.mult, op1=ALU.add)
        mx = sb.tile([N, 8], F32, tag="topk_mx")
        done = 0
        while done < nb:
            kk = min(8, nb - done)
            nc.vector.max(out=mx[:], in_=x[:])
            if kk < 8:
                nc.vector.memset(mx[:, kk:], 0.0)
            nc.vector.match_replace(out=x[:], in_to_replace=mx[:],
                                    in_values=x[:], imm_value=0.0)
            done += kk
        m = sb.tile([N, N], F16, tag="topk_m")
        nc.vector.tensor_scalar(out=m[:], in0=x[:], scalar1=0.5, scalar2=None,
                                op0=ALU.is_gt)
        em = sb.tile([N, N], F16, tag=f"em_{stage}")
        nc.vector.scalar_tensor_tensor(out=em[:], in0=e[:], scalar=1.0,
                                       op0=ALU.bypass, in1=m[:], op1=ALU.mult,
                                       accum_out=s1[:])
    else:
        em = e
        nc.vector.scalar_tensor_tensor(out=e[:], in0=th[:], scalar=1.0,
                                       op0=ALU.add, in1=ep_nd_half[:],
                                       op1=ALU.mult, accum_out=s1[:])
    # v = shalf/(s1*rs) where rs = 2*shalf + RS_CONST;  f = 2*v
    rinv = sb.tile([N, 1], F32, tag="rinv")
    nc.vector.reciprocal(rinv[:], s1[:])
    t1 = sb.tile([N, 1], F32, tag="vt1")
    nc.vector.tensor_tensor(out=t1[:], in0=shalf[:], in1=rinv[:], op=ALU.mult)
    v = sb.tile([N, 1], F32, tag="v")
    nc.vector.tensor_tensor(out=v[:], in0=t1[:], in1=r2[:], op=ALU.mult)
    return em, v


def _emit_mha_scores(nc, P, sb, g, vpT):
    pqk = P.aux.tile([128, 512], F32, tag="pz")
    nc.tensor.matmul(pqk[:, 0:128], P.s16(f"wq_{g}")[:], vpT[:],
                     start=True, stop=True)
    nc.tensor.matmul(pqk[:, 128:256], P.s16(f"wk_{g}")[:], vpT[:],
                     start=True, stop=True)
    qkT = sb.tile([C, 2 * N], F16, tag=f"qkT_{g}")
    nc.vector.tensor_scalar(out=qkT[:], in0=pqk[:, 0:256], scalar1=1.0,
                            scalar2=None, op0=ALU.mult)
    qkh = sb.tile([DK, H, 2 * N], F16, tag=f"qkh_{g}")
    for h in range(H):
        nc.sync.dma_start(qkh[0:DK, h, :], qkT[DK * h:DK * (h + 1), :])
    ps2 = P.aux.tile([128, 512], F32, tag="pz")
    for h in range(H):
        nc.tensor.matmul(ps2[:, 128 * h:128 * (h + 1)],
                         qkh[0:DK, h, 0:N], qkh[0:DK, h, N:2 * N],
                         start=True, stop=True)
    ssb = sb.tile([128, 512], F16, tag=f"mha_s_{g}")
    nc.vector.tensor_scalar(out=ssb[:], in0=ps2[:], scalar1=1.0, scalar2=None,
                            op0=ALU.mult)
    inv_sqrt_dk = float(1.0 / np.sqrt(DK))
    mxs = sb.tile([N, H], F32, tag=f"mha_mx_{g}")
    nbias = sb.tile([N, H], F32, tag=f"mha_nb_{g}")
    nc.vector.tensor_reduce(
        mxs[:], ssb[:].rearrange("p (h n) -> p h n", h=H),
        axis=mybir.AxisListType.X, op=ALU.max)
    nc.vector.tensor_scalar(out=nbias[:], in0=mxs[:], scalar1=-inv_sqrt_dk,
                            scalar2=None, op0=ALU.mult)
    return ssb, nbias


def _emit_mha_softmax(nc, P, sb, g, ssb, nbias):
    inv_sqrt_dk = float(1.0 / np.sqrt(DK))
    ehs, ses = [], []
    for h in range(H):
        eh = sb.tile([N, N], F16, tag=f"mha_eh{h}")
        se = sb.tile([N, 1], F32, tag=f"mha_se{h}")
        nc.scalar.activation(eh[:], ssb[:, 128 * h:128 * (h + 1)], ACT.Exp,
                             bias=nbias[:, h:h + 1], scale=inv_sqrt_dk,
                             accum_out=se[:])
        ehs.append(eh)
        ses.append(se)
    rcs = []
    for h in range(H):
        rc = sb.tile([N, 1], F32, tag=f"mha_rc{h}")
        nc.vector.reciprocal(rc[:], ses[h][:])
        rcs.append(rc)
    t1 = sb.tile([N, N], F16, tag="mha_t1")
    nc.vector.tensor_scalar(out=t1[:], in0=ehs[1][:], scalar1=rcs[1][:],
                            scalar2=None, op0=ALU.mult)
    a01 = sb.tile([N, N], F16, tag="mha_a01")
    nc.vector.scalar_tensor_tensor(out=a01[:], in0=ehs[0][:], scalar=rcs[0][:],
                                   op0=ALU.mult, in1=t1[:], op1=ALU.add)
    t2 = sb.tile([N, N], F16, tag="mha_t2")
    nc.vector.tensor_scalar(out=t2[:], in0=ehs[3][:], scalar1=rcs[3][:],
                            scalar2=None, op0=ALU.mult)
    a23 = sb.tile([N, N], F16, tag="mha_a23")
    nc.vector.scalar_tensor_tensor(out=a23[:], in0=ehs[2][:], scalar=rcs[2][:],
                                   op0=ALU.mult, in1=t2[:], op1=ALU.add)
    attn = sb.tile([N, N], F16, tag=f"attn_{g}")
    nc.vector.tensor_tensor(out=attn[:], in0=a01[:], in1=a23[:], op=ALU.add)
    # no diagonal kill needed: em's diagonal is 0, so edge = em*attn has it too
    return attn


def _emit_d2p(nc, P, sb, g, em, attn, vpT, vp_rm, last):
    """vpT_new = D2PAgg(em-normalized edges, vp).

    Returns (vpT_new f16, vp_rm_new or None)."""
    edge = sb.tile([N, N], F16, tag="edge")
    s = sb.tile([N, 1], F32, tag="d2p_s")
    nc.vector.scalar_tensor_tensor(out=edge[:], in0=em[:], scalar=1.0,
                                   op0=ALU.bypass, in1=attn[:], op1=ALU.mult,
                                   accum_out=s[:])
    rr = sb.tile([N, 1], F32, tag="d2p_rr")
    nc.vector.reciprocal(rr[:], s[:])
    diagm = sb.tile([N, N], F16, tag="diagm")
    nc.vector.tensor_scalar(out=diagm[:], in0=P.ident16[:], scalar1=rr[:],
                            scalar2=None, op0=ALU.mult)
    pTe = P.aux.tile([128, 512], F32, tag="pz")
    nc.tensor.matmul(pTe[:, 0:128], edge[:], diagm[:], start=True, stop=True)
    edgeTs = sb.tile([N, N], F16, tag="edgeTs")
    nc.vector.tensor_scalar(out=edgeTs[:], in0=pTe[:, 0:128], scalar1=1.0,
                            scalar2=None, op0=ALU.mult)
    pa = P.aux.tile([128, 512], F32, tag="pz")
    nc.tensor.matmul(pa[:, 0:128], vp_rm[:], edgeTs[:], start=True, stop=True)
    aggrT = sb.tile([C, N], F16, tag="aggrT")
    nc.vector.tensor_scalar(out=aggrT[:], in0=pa[:, 0:128], scalar1=1.0,
                            scalar2=None, op0=ALU.mult)
    hs = []
    for oh in range(2):
        pm = P.aux.tile([128, 512], F32, tag="pz")
        nc.tensor.matmul(pm[:, 0:128],
                         P.s16(f"d1k0_{g}")[:, C * oh:C * (oh + 1)], vpT[:],
                         start=True, stop=False)
        nc.tensor.matmul(pm[:, 0:128],
                         P.s16(f"d1k1_{g}")[:, C * oh:C * (oh + 1)], aggrT[:],
                         start=False, stop=True)
        h_oh = sb.tile([C, N], F16, tag=f"d2ph{oh}")
        nc.scalar.activation(h_oh[:], pm[:, 0:128], ACT.Prelu, alpha=0.01)
        hs.append(h_oh)
    pv = P.aux.tile([128, 512], F32, tag="pz")
    nc.tensor.matmul(pv[:, 0:128], P.s16(f"d2k0_{g}")[:], hs[0][:],
                     start=True, stop=False)
    nc.tensor.matmul(pv[:, 0:128], P.s16(f"d2k1_{g}")[:], hs[1][:],
                     start=False, stop=True)
    vpT_new = sb.tile([C, N], F16, tag=f"vpT_{g + 1}")
    nc.scalar.activation(vpT_new[:], pv[:, 0:128], ACT.Prelu, alpha=0.01)
    vp_rm_new = None
    if not last:
        pt2 = P.aux.tile([128, 1024], F16, tag="pz")
        nc.tensor.transpose(pt2[:, 0:128], vpT_new[:], P.ident16[:])
        vp_rm_new = sb.tile([N, C], F16, tag=f"vprm_{g + 1}")
        nc.vector.tensor_scalar(out=vp_rm_new[:], in0=pt2[:, 0:128],
                                scalar1=1.0, scalar2=None, op0=ALU.mult)
    return vpT_new, vp_rm_new


def build():
    """Build the single-core Bass graph (SPMD across 8 cores)."""
    nc = bass.Bass()
    c16_ext = nc.declare_dram_parameter("core16", [128, TOTC16], F16,
                                        isOutput=False)
    b16_ext = nc.declare_dram_parameter("blob16", [128, TOT16], F16,
                                        isOutput=False)
    c32_ext = nc.declare_dram_parameter("core32", [128, N], F32,
                                        isOutput=False)
    b32_ext = nc.declare_dram_parameter("blob32", [128, TOT32], F32,
                                        isOutput=False)
    mask_ext = nc.declare_dram_parameter("masklow", [N, N], U8, isOutput=False)
    out_ext = nc.declare_dram_parameter("out", [N, N], F32, isOutput=True)

    with tile_mod.TileContext(nc) as tc:
        with tc.tile_pool(name="const", bufs=1) as const, \
             tc.tile_pool(name="state", bufs=1) as state, \
             tc.tile_pool(name="work", bufs=3) as work, \
             tc.tile_pool(name="p1", bufs=2, space="PSUM") as p1, \
             tc.tile_pool(name="p2", bufs=2, space="PSUM") as p2, \
             tc.tile_pool(name="pz", bufs=2, space="PSUM") as pz:

            P = _P()
            P.work, P.p1, P.p2, P.aux = work, p1, p2, pz

            c16 = const.tile([128, TOTC16], F16, tag="c16")
            nc.sync.dma_start(c16[:], c16_ext[:])
            b16 = const.tile([128, TOT16], F16, tag="b16")
            nc.sync.dma_start(b16[:], b16_ext[:])
            c32 = const.tile([128, N], F32, tag="c32")
            nc.gpsimd.dma_start(c32[:], c32_ext[:])
            b32 = const.tile([128, TOT32], F32, tag="b32")
            nc.gpsimd.dma_start(b32[:], b32_ext[:])
            P.masklow = const.tile([N, N], U8, tag="masklow")
            nc.gpsimd.dma_start(P.masklow[:], mask_ext[:])

            def s16(name):
                off, wd = OFF16[name]
                return b16[:, off:off + wd]

            def s32(name):
                off, wd = OFF32[name]
                return b32[:, off:off + wd]

            P.s16, P.s32 = s16, s32
            P.ident16 = s16("ident16")
            P.omeye16 = s16("omeye16")

            vpT0 = c16[:, OFFC16["vpT0"][0]:OFFC16["vpT0"][0] + N]
            vprm0 = c16[:, OFFC16["vprm0"][0]:OFFC16["vprm0"][0] + C]
            ep0 = c32[:]

            # ---------------- stage pre ----------------
            zsb = state.tile([N, N], F16, tag="zsb_pre")
            ep_nd_half = state.tile([N, N], F16, tag="ephalf_pre")
            shalf = state.tile([N, 1], F32, tag="shalf_pre")
            r2 = state.tile([N, 1], F32, tag="r2_pre")

            scores = {}

            def pre_hook2():
                # ep-derived context (waits on the later c32 DMA; emitted
                # after chunk 2 to avoid head-of-line blocking the DVE)
                nc.vector.scalar_tensor_tensor(
                    out=ep_nd_half[:], in0=ep0[:], scalar=0.5, op0=ALU.mult,
                    in1=P.omeye16[:], op1=ALU.mult, accum_out=shalf[:])
                rr = P.work.tile([N, 1], F32, tag="rrtmp")
                nc.vector.tensor_scalar(out=rr[:], in0=shalf[:], scalar1=2.0,
                                        scalar2=RS_CONST, op0=ALU.mult,
                                        op1=ALU.add)
                nc.vector.reciprocal(r2[:], rr[:])

            def pre_hook4():
                scores[0] = _emit_mha_scores(nc, P, state, 0, vpT0)

            def pre_hook10():
                scores["attn0"] = _emit_mha_softmax(nc, P, state, 0,
                                                    *scores[0])

            _emit_psim_chunks(nc, P, state, "pre", vpT0, zsb,
                              {2: pre_hook2, 4: pre_hook4, 10: pre_hook10})
            em, v = _emit_epilogue(nc, P, state, "pre", zsb,
                                   (ep_nd_half, shalf, r2), 0)

            vpT1, vprm1 = _emit_d2p(nc, P, state, 0, em, scores["attn0"],
                                    vpT0, vprm0, last=False)
            # next-stage ep context from em (off critical path)
            ctx_g0 = _next_ctx(nc, P, state, "g0", em, v)

            # ---------------- stage g0 ----------------
            zsb1 = state.tile([N, N], F16, tag="zsb_g0")

            def g0_hook4():
                scores[1] = _emit_mha_scores(nc, P, state, 1, vpT1)

            def g0_hook10():
                scores["attn1"] = _emit_mha_softmax(nc, P, state, 1,
                                                    *scores[1])

            _emit_psim_chunks(nc, P, state, "g0", vpT1, zsb1,
                              {4: g0_hook4, 10: g0_hook10})
            nb0 = N - int(N * 0.9)
            em0, v0 = _emit_epilogue(nc, P, state, "g0", zsb1, ctx_g0, nb0)
            vpT2, _ = _emit_d2p(nc, P, state, 1, em0, scores["attn1"],
                                vpT1, vprm1, last=True)
            ctx_g1 = _next_ctx(nc, P, state, "g1", em0, v0)

            # ---------------- stage g1 ----------------
            zsb2 = state.tile([N, N], F16, tag="zsb_g1")

            eyeg = state.tile([N, N], F32, tag="eyeg")

            def g1_hook4():
                nc.vector.tensor_scalar(out=eyeg[:], in0=P.s32("eye_eps")[:],
                                        scalar1=ctx_g1[2][:], scalar2=None,
                                        op0=ALU.mult)

            _emit_psim_chunks(nc, P, state, "g1", vpT2, zsb2, {4: g1_hook4})
            nb1 = N - int(N * 0.8)
            em1, v1 = _emit_epilogue(nc, P, state, "g1", zsb2, ctx_g1, nb1)
            f = state.tile([N, 1], F32, tag="ffin")
            nc.vector.tensor_scalar(out=f[:], in0=v1[:], scalar1=2.0,
                                    scalar2=None, op0=ALU.mult)
            outt = state.tile([N, N], F32, tag="outt")
            nc.vector.scalar_tensor_tensor(out=outt[:], in0=em1[:],
                                           scalar=f[:], op0=ALU.mult,
                                           in1=eyeg[:], op1=ALU.add)
            nc.sync.dma_start(out_ext[:], outt[:])

    return nc


def _next_ctx(nc, P, state, stage, em, v):
    """ep context for the next psim stage, derived from em and v = f/2."""
    ephn = state.tile([N, N], F16, tag=f"ephalf_{stage}")
    shn = state.tile([N, 1], F32, tag=f"shalf_{stage}")
    nc.vector.tensor_scalar(out=ephn[:], in0=em[:], scalar1=v[:],
                            scalar2=0.0, op0=ALU.mult, op1=ALU.add,
                            accum_out=shn[:])
    rr = state.tile([N, 1], F32, tag=f"rr_{stage}")
    nc.vector.tensor_scalar(out=rr[:], in0=shn[:], scalar1=2.0,
                            scalar2=RS_CONST, op0=ALU.mult, op1=ALU.add)
    r2n = state.tile([N, 1], F32, tag=f"r2_{stage}")
    nc.vector.reciprocal(r2n[:], rr[:])
    return (ephn, shn, r2n)


def make_in_maps(vp, ep0, ps_pre_w1, ps_pre_w2, ps_pre_w3, ps_pre_b3,
                 ps_w1, ps_w2, ps_w3, ps_b3, d2p_w1, d2p_w2, mha_wq, mha_wk):
    f = np.float32
    a = lambda x: np.ascontiguousarray(np.asarray(x), dtype=f)
    eye = np.eye(N, dtype=f)
    ii = np.arange(N)
    masklow = np.ascontiguousarray(
        (ii[None, :] < (ii[:, None] // _BLK) * _BLK).astype(np.uint8))

    vals16 = {"ident16": eye, "omeye16": a(1.0 - eye)}
    vals32 = {"eye_eps": a(eye + 1e-6)}
    for s, w1, w2, w3, b3 in [
        ("pre", ps_pre_w1, ps_pre_w2, ps_pre_w3, ps_pre_b3),
        ("g0", ps_w1[0], ps_w2[0], ps_w3[0], ps_b3[0]),
        ("g1", ps_w1[1], ps_w2[1], ps_w3[1], ps_b3[1]),
    ]:
        w2s = a(np.asarray(w2) * BN_SCALE)
        vals16[f"w1_{s}"] = a(np.asarray(w1) * BN_SCALE)
        vals16[f"w2a_{s}"] = w2s[:C]
        vals16[f"w2b_{s}"] = w2s[C:]
        vals16[f"w3_{s}"] = a(w3)
        vals32[f"b3h_{s}"] = a(np.broadcast_to(
            np.asarray(b3).reshape(1, 1) * 0.5, (N, 1)))
    for g in range(G):
        d1s = a(np.asarray(d2p_w1[g]) * BN_SCALE)   # [2C, 2C]
        d2s = a(np.asarray(d2p_w2[g]) * BN_SCALE)   # [2C, C]
        vals16[f"d1k0_{g}"] = d1s[:C]
        vals16[f"d1k1_{g}"] = d1s[C:]
        vals16[f"d2k0_{g}"] = d2s[:C]
        vals16[f"d2k1_{g}"] = d2s[C:]
        vals16[f"wq_{g}"] = a(mha_wq[g])
        vals16[f"wk_{g}"] = a(mha_wk[g])

    blob16 = np.zeros((128, TOT16), dtype=np.float16)
    for name, wd in BLOB16:
        off = OFF16[name][0]
        blob16[:, off:off + wd] = vals16[name].astype(np.float16)
    blob32 = np.zeros((128, TOT32), dtype=f)
    for name, wd in BLOB32:
        off = OFF32[name][0]
        blob32[:, off:off + wd] = vals32[name]

    shared = {"blob16": blob16, "blob32": blob32, "masklow": masklow}
    vp = a(vp)
    ep0 = a(ep0)
    in_maps = []
    for i in range(B):
        c16 = np.zeros((128, TOTC16), dtype=np.float16)
        c16[:, OFFC16["vpT0"][0]:OFFC16["vpT0"][0] + N] = vp[i].T
        c16[:, OFFC16["vprm0"][0]:OFFC16["vprm0"][0] + C] = vp[i]
        m = dict(shared)
        m["core16"] = c16
        m["core32"] = ep0[i]
        in_maps.append(m)
    return in_maps


_CACHED_NC = None


def _get_nc():
    global _CACHED_NC
    if _CACHED_NC is None:
        _CACHED_NC = build()
    return _CACHED_NC


def run(in_maps, trace=False):
    from concourse.bass_utils import run_bass_kernel_spmd

    nc = _get_nc()
    return run_bass_kernel_spmd(nc, in_maps, list(range(B)), trace=trace)


def kernel(**inputs) -> np.ndarray:
    in_maps = make_in_maps(**inputs)
    r = run(in_maps, trace=False)
    return np.stack([r.results[i]["out"] for i in range(B)]).astype(np.float32)
